# revision 16
# baseline (speedup 1.0000x reference)
"""Trainium2 Bass kernel v2 for MultiHeadSelfAttention (RMSNorm + QKV + causal SDPA + out-proj).

Sharding: 8 cores = batch(2) x head-groups(4); each core: one batch element,
4 of 16 heads (EL=512 d-slice), out-proj partial over its slice, host sums.

v2 changes vs baseline:
  - all matmul operands bf16 (host-cast); fp32 PSUM accumulation. Same PE
    cycles/row as fp32r at >=256 width, but no narrow-tile 4x penalty, half
    the DMA bytes and SBUF footprint.
  - Q/K/V stay SBUF-resident between projection and attention (no DRAM
    round-trip).
  - softmax denominator comes FREE from the AV matmul: AV runs with pt as
    the stationary operand per 128-q block against v_ext [k, dh+1] whose
    last column is ones, so PSUM col 128 accumulates z[q] in exactly the
    per-q-partition orientation the normalize needs (tensor_scalar).
    The old ones-row z matmul (29us of PE) is gone.
  - normalized y [q, dh] blocks are PE-transposed back to [dh, q] for the
    out-projection (cheap: 128 cols each).
  - RMSNorm: norm_weight folded into weights on host; x normalized once
    (xn = x * rsqrt(mean x^2)) on DVE, so q/k/v need no post-scales.
"""

import sys

sys.path.insert(0, '/opt/trn_rl_repo')

import numpy as np
import ml_dtypes

import concourse.bass as bass  # noqa: F401  (import order matters)
from concourse import bacc
import concourse.mybir as mybir
import concourse.tile as tile
from concourse.bass_utils import run_bass_kernel_spmd

B, T, D = 2, 2048, 2048
H_TOT, H_LOC, DH = 16, 4, 128
EL = H_LOC * DH            # 512: local q/k/v width
ND = D // 128              # 16 d-tiles
NT = T // 128              # 16 t-tiles
CH = 512                   # token chunk
NCH = T // CH              # 4 chunks
QT = CH // 128             # 4 q-tiles per chunk
VW = 132                   # v row width: 128 dh + ones col + pad
EPS = 1e-6
F32 = mybir.dt.float32
BF16 = mybir.dt.bfloat16
F8 = mybir.dt.float8e4
X_S = 8.0            # xh = fp8(x * X_S), xl = fp8((x - xh/X_S) * X_S)
W_S = 128.0          # wh = fp8(W * W_S), wl = fp8((W - wh/W_S) * W_S)
P_S = X_S * W_S      # all three residual products share scale P_S = 1024,
                     # so xh@wh + xl@wh + xh@wl accumulate in ONE psum group;
                     # the 1/P_S dequant rides inside the rmsnorm scale
MULT = mybir.AluOpType.mult
SC = float(1.0 / np.sqrt(DH))


def _build():
    nc = bacc.Bacc("TRN2")
    xhT = nc.dram_tensor("xhT", [D, T], F8, kind="ExternalInput")
    xlT = nc.dram_tensor("xlT", [D, T], F8, kind="ExternalInput")
    wqkhT = nc.dram_tensor("wqkhT", [D, 2 * EL], F8, kind="ExternalInput")
    wqklT = nc.dram_tensor("wqklT", [D, 2 * EL], F8, kind="ExternalInput")
    wvhT = nc.dram_tensor("wvhT", [D, EL], F8, kind="ExternalInput")
    wvlT = nc.dram_tensor("wvlT", [D, EL], F8, kind="ExternalInput")
    woutT = nc.dram_tensor("woutT", [EL, D], BF16, kind="ExternalInput")
    mask = nc.dram_tensor("mask", [128, 128], BF16, kind="ExternalInput")
    eye = nc.dram_tensor("eye", [128, 128], BF16, kind="ExternalInput")
    outT = nc.dram_tensor("outT", [D, T], BF16, kind="ExternalOutput")

    with tile.TileContext(nc) as tc:
        with tc.tile_pool(name="const", bufs=1) as cst, \
             tc.tile_pool(name="dram", bufs=1, space="DRAM") as dramp, \
             tc.tile_pool(name="persist", bufs=1) as pers:
            mask_sb = cst.tile([128, 128], BF16)
            eye_sb = cst.tile([128, 128], BF16)
            ones = cst.tile([128, 8], BF16)
            nc.gpsimd.memset(ones[:], 1.0)
            eps_sb = cst.tile([1, 1], F32)
            nc.gpsimd.memset(eps_sb[:], EPS * P_S * P_S)

            wqkh_sb = pers.tile([128, ND // 2, 2, 2 * EL], F8)
            wqkl_sb = pers.tile([128, ND // 2, 2, 2 * EL], F8)
            wvh_sb = pers.tile([128, ND // 2, 2, EL], F8)
            wvl_sb = pers.tile([128, ND // 2, 2, EL], F8)
            wout_sb = pers.tile([128, H_LOC, D], BF16)
            qT_sb = pers.tile([128, H_LOC, T], BF16)
            kT_sb = pers.tile([128, H_LOC, T], BF16)
            v_sb = pers.tile([128, NT, H_LOC, VW], BF16)
            nc.gpsimd.memset(v_sb[:, :, :, 128:129], 1.0)

            def load_weights_qk():
                # et-pair granularity so QKV(0) can start before the full hi set lands
                for g in range(4):
                    nc.sync.dma_start(
                        wqkh_sb[:, :, :, g * 256:(g + 1) * 256],
                        wqkhT[0:D, g * 256:(g + 1) * 256].rearrange("(i two p) e -> p i two e", p=128, two=2))

            def load_weights_rest():
                nc.sync.dma_start(wqkl_sb[:], wqklT[0:D, :].rearrange("(i two p) e -> p i two e", p=128, two=2))
                nc.sync.dma_start(wvh_sb[:], wvhT[0:D, :].rearrange("(i two p) e -> p i two e", p=128, two=2))
                nc.sync.dma_start(wvl_sb[:], wvlT[0:D, :].rearrange("(i two p) e -> p i two e", p=128, two=2))

            def load_wout():
                nc.sync.dma_start(wout_sb[:], woutT[0:EL, :].rearrange("(dl p) e -> p dl e", p=128))
                nc.sync.dma_start(mask_sb[:], mask[:, :])
                nc.sync.dma_start(eye_sb[:], eye[:, :])

            # ---------------- Phase A: RMSNorm + QKV projection ----------------
            with tc.tile_pool(name="xa", bufs=2) as xa_p, \
                 tc.tile_pool(name="pa_sb", bufs=2) as pa_sb, \
                 tc.tile_pool(name="pa_ps", bufs=2, space="PSUM") as pa_ps, \
                 tc.tile_pool(name="pa_ps1", bufs=1, space="PSUM") as pa_ps1:
                xc_tiles = {}
                xn_tiles = {}
                sq_tiles = {}
                sbc_tiles = {}

                def emit_load(c, parts=(ND // 2,)):
                    xh = xa_p.tile([128, ND // 2, 2, CH], F8, tag="xh")
                    xl = xa_p.tile([128, ND // 2, 2, CH], F8, tag="xl")
                    k0 = 0
                    for n in parts:
                        nc.sync.dma_start(
                            xh[:, k0:k0 + n, :, :],
                            xhT[k0 * 256:(k0 + n) * 256,
                                c * CH:(c + 1) * CH].rearrange("(i two p) t -> p i two t", p=128, two=2))
                        k0 += n
                    nc.sync.dma_start(
                        xl[:, :, :, :],
                        xlT[0:D, c * CH:(c + 1) * CH].rearrange("(i two p) t -> p i two t", p=128, two=2))
                    xc_tiles[c] = (xh, xl)

                def emit_sq(c):
                    xh, _xl = xc_tiles[c]
                    sqs = []
                    for kd in range(ND):
                        sq = pa_sb.tile([128, CH], BF16, tag="sq", bufs=4)
                        src = xh[:, kd // 2, kd % 2, :]
                        nc.vector.tensor_tensor(sq[:], src, src, MULT)
                        sqs.append(sq)
                    sq_tiles[c] = sqs

                def emit_ssq_chain(c, preload_exp=False):
                    sqs = sq_tiles.pop(c)
                    ssq = pa_ps1.tile([1, CH], F32, tag="ssq")
                    for kd in range(ND):
                        nc.tensor.matmul(ssq[:], ones[:, 0:1], sqs[kd][:], start=(kd == 0), stop=(kd == ND - 1))
                    tmp_s = pa_sb.tile([1, CH], F32, tag="tmp_s")
                    nc.scalar.activation(tmp_s[:], ssq[:], mybir.ActivationFunctionType.Sqrt,
                                         bias=eps_sb[:], scale=P_S * P_S / (X_S * X_S * D))
                    if preload_exp:
                        # pull the Exp act-table load into Phase A slack so the
                        # first Phase B exp doesn't pay the 1.3us switch
                        dummy = pa_sb.tile([1, 1], F32, tag="expdummy")
                        nc.scalar.activation(dummy[:], eps_sb[:],
                                             mybir.ActivationFunctionType.Exp)
                    s_row = pa_sb.tile([1, CH], F32, tag="s_row")
                    nc.vector.reciprocal(s_row[:], tmp_s[:])
                    s_bf = pa_sb.tile([1, CH], BF16, tag="s_bf")
                    nc.vector.tensor_copy(s_bf[:], s_row[:])
                    sb_c = pa_sb.tile([128, CH], BF16, tag="sb_c")
                    nc.gpsimd.partition_broadcast(sb_c[:], s_bf[:])
                    # SBUF->SBUF scatter DMA misbehaves on hw; bounce via DRAM
                    s_d = dramp.tile([1, CH], F32, tag="s_d", bufs=2)
                    nc.sync.dma_start(s_d[:], s_row[:])
                    s_col = pa_sb.tile([128, QT], F32, tag="s_col")
                    nc.sync.dma_start(s_col[:], s_d[0:1, :].rearrange("o (tt p) -> p (o tt)", p=128))
                    sbc_tiles[c] = (sb_c, s_col)

                def emit_qkv(c, mid_hook=None, tail_hook=None):
                    xh, xl = xc_tiles.pop(c)
                    sb_c, s_col = sbc_tiles.pop(c)
                    DR = mybir.MatmulPerfMode.DoubleRow
                    NP = ND // 2

                    def residual_mm(g, et_sl, moving_of):
                        for i in range(NP):
                            nc.tensor.matmul(g, wqkh_sb[:, i, :, et_sl], moving_of(xh, i),
                                             perf_mode=DR, start=(i == 0), stop=False)
                        for i in range(NP):
                            nc.tensor.matmul(g, wqkh_sb[:, i, :, et_sl], moving_of(xl, i),
                                             perf_mode=DR, start=False, stop=False)
                        for i in range(NP):
                            nc.tensor.matmul(g, wqkl_sb[:, i, :, et_sl], moving_of(xh, i),
                                             perf_mode=DR, start=False, stop=(i == NP - 1))

                    for et in range(8):   # 4 q tiles then 4 k tiles
                        if et == 4 and mid_hook is not None:
                            mid_hook()
                        esl = slice(et * 128, (et + 1) * 128)
                        g = pa_ps.tile([128, CH], F32, tag="g", bufs=3)
                        residual_mm(g[:], esl, lambda t, i: t[:, i, :, :])
                        dst = qT_sb if et < 4 else kT_sb
                        nc.vector.tensor_tensor(dst[:, et % 4, c * CH:(c + 1) * CH], g[:], sb_c[:], MULT)
                    for tt in range(QT):
                        j = c * QT + tt
                        tsl = slice(tt * 128, (tt + 1) * 128)
                        g = pa_ps.tile([128, EL], F32, tag="g", bufs=3)
                        for i in range(NP):
                            nc.tensor.matmul(g[:], xh[:, i, :, tsl], wvh_sb[:, i, :, :],
                                             perf_mode=DR, start=(i == 0), stop=False)
                        for i in range(NP):
                            nc.tensor.matmul(g[:], xl[:, i, :, tsl], wvh_sb[:, i, :, :],
                                             perf_mode=DR, start=False, stop=False)
                        for i in range(NP):
                            nc.tensor.matmul(g[:], xh[:, i, :, tsl], wvl_sb[:, i, :, :],
                                             perf_mode=DR, start=False, stop=(i == NP - 1))
                        for h in range(H_LOC):
                            if c == NCH - 1:
                                nc.vector.tensor_scalar_mul(v_sb[:, j, h, 0:128],
                                                            g[:, h * 128:(h + 1) * 128],
                                                            s_col[:, tt:tt + 1])
                            else:
                                nc.scalar.activation(v_sb[:, j, h, 0:128], g[:, h * 128:(h + 1) * 128],
                                                     mybir.ActivationFunctionType.Copy,
                                                     scale=s_col[:, tt:tt + 1])
                    if tail_hook is not None:
                        tail_hook()

                emit_load(0, parts=(1, 1, 2, 4))
                load_weights_qk()
                load_weights_rest()
                emit_load(1)
                load_wout()
                emit_sq(0)
                emit_ssq_chain(0)
                for c in range(NCH):
                    def mid():
                        if c + 1 < NCH:
                            emit_ssq_chain(c + 1, preload_exp=(c + 1 == NCH - 1))
                            if c + 2 < NCH:
                                emit_load(c + 2)

                    def tail():
                        pass
                    if c + 1 < NCH:
                        emit_sq(c + 1)
                    emit_qkv(c, mid_hook=mid, tail_hook=tail)

            # ---------------- Phase B: causal attention + out-projection ----------------
            with tc.tile_pool(name="pb_sb", bufs=2) as pb_sb, \
                 tc.tile_pool(name="pb_ps", bufs=2, space="PSUM") as pb_ps:

                def emit_scores(c, h):
                    """score matmuls + exp + mask for (chunk c, head h); returns pt tiles."""
                    jmax = (c + 1) * QT
                    pts = []
                    for j in range(jmax):
                        off = (j - c * QT) * 128 if j >= c * QT else 0
                        st_ps = pb_ps.tile([128, CH], F32, tag="st_ps", bufs=3)
                        nc.tensor.matmul(st_ps[:, off:], kT_sb[:, h, j * 128:(j + 1) * 128],
                                         qT_sb[:, h, c * CH + off:(c + 1) * CH], start=True, stop=True)
                        pt = pb_sb.tile([128, CH], BF16, tag="pt", bufs=20)
                        nc.scalar.activation(pt[:, off:], st_ps[:, off:],
                                             mybir.ActivationFunctionType.Exp, scale=SC)
                        if j >= c * QT:
                            nc.vector.tensor_tensor(pt[:, off:off + 128], pt[:, off:off + 128],
                                                    mask_sb[:], MULT)
                        pts.append(pt)
                    return pts

                def emit_av(c, h, pts, y_sb):
                    """AV with stationary pt per q-block; v_ext ones col gives z in PSUM
                    col 128; then normalize on DVE. Returns yn tiles for later transpose."""
                    yT_a = pb_ps.tile([128, 2, 129], F32, tag="yT_a", bufs=1)
                    yT_b = pb_ps.tile([128, 2, 129], F32, tag="yT_b", bufs=1)
                    halves = [yT_a, yT_b]
                    # accumulate with start=False onto memset-zeroed PSUM: four
                    # interleaved q-block groups share banks, and start=True would
                    # zero the whole 2KB zero-region (clobbering neighbors)
                    nc.vector.memset(yT_a[:], 0.0)
                    nc.vector.memset(yT_b[:], 0.0)
                    for j in range((c + 1) * QT):
                        off = (j - c * QT) * 128 if j >= c * QT else 0
                        pt = pts[j]
                        for qb in range(QT):
                            if qb * 128 < off:
                                continue
                            nc.tensor.matmul(halves[qb // 2][:, qb % 2, :],
                                             pt[:, qb * 128:(qb + 1) * 128],
                                             v_sb[:, j, h, 0:129],
                                             start=False, stop=(j == c * QT + qb),
                                             skip_group_check=True)
                    yns = []
                    for qb in range(QT):
                        yT = halves[qb // 2][:, qb % 2, :]
                        rz = pb_sb.tile([128, 1], F32, tag="rz", bufs=4)
                        nc.vector.reciprocal(rz[:], yT[:, 128:129])
                        yn = pb_sb.tile([128, 128], BF16, tag="yn", bufs=8)
                        nc.vector.tensor_scalar_mul(yn[:], yT[:, 0:128], rz[:])
                        yns.append(yn)
                    return yns

                def emit_transposes(c, h, yns, y_sb):
                    for qb in range(QT):
                        tr_ps = pb_ps.tile([128, 128], BF16, tag="tr_ps", bufs=1)
                        nc.tensor.transpose(tr_ps[:], yns[qb][:], eye_sb[:])
                        nc.vector.tensor_copy(y_sb[:, h, qb * 128:(qb + 1) * 128], tr_ps[:])

                def emit_outproj_part(c, y_sb, eo_list):
                    for eo in eo_list:
                        o_ps = pb_ps.tile([128, CH], F32, tag="o_ps", bufs=2)
                        for dl in range(H_LOC):
                            nc.tensor.matmul(o_ps[:], wout_sb[:, dl, eo * 128:(eo + 1) * 128],
                                             y_sb[:, dl, :], start=(dl == 0), stop=(dl == H_LOC - 1))
                        o_sb = pb_sb.tile([128, CH], BF16, tag="o_sb", bufs=4)
                        if eo % 2 == 0:
                            nc.vector.tensor_copy(o_sb[:], o_ps[:])
                        else:
                            nc.scalar.activation(o_sb[:], o_ps[:], mybir.ActivationFunctionType.Copy)
                        nc.sync.dma_start(outT[eo * 128:(eo + 1) * 128, c * CH:(c + 1) * CH], o_sb[:])

                # slot schedule per chunk: scores(h) | AV(h-1)+norm | transposes(h-2)
                # | previous chunk's out-proj spread across slots
                prev_y = None
                for c in range(NCH):
                    y_sb = pb_sb.tile([128, H_LOC, CH], BF16, tag="y_sb", bufs=2)
                    pend_av = None   # (h, pts)
                    pend_tr = None   # (h, yns)
                    for h in range(H_LOC):
                        pts = emit_scores(c, h)
                        if prev_y is not None:
                            eos = [range(0, 8), range(8, 12), range(12, 16), range(0, 0)][h]
                            emit_outproj_part(c - 1, prev_y, eos)
                        if pend_av is not None:
                            yns = emit_av(c, pend_av[0], pend_av[1], y_sb)
                            if pend_tr is not None:
                                emit_transposes(c, pend_tr[0], pend_tr[1], y_sb)
                            pend_tr = (pend_av[0], yns)
                        pend_av = (h, pts)
                    yns = emit_av(c, pend_av[0], pend_av[1], y_sb)
                    if pend_tr is not None:
                        emit_transposes(c, pend_tr[0], pend_tr[1], y_sb)
                    emit_transposes(c, pend_av[0], yns, y_sb)
                    prev_y = y_sb
                emit_outproj_part(NCH - 1, prev_y, range(16))
    nc.finalize()
    return nc


_BUILT = None


def _get_nc():
    global _BUILT
    if _BUILT is None:
        _BUILT = _build()
    return _BUILT


def _bf(a):
    return np.ascontiguousarray(a).astype(ml_dtypes.bfloat16)


def _hilo(a, s_hi, s_lo):
    hi = np.ascontiguousarray(a * s_hi).astype(ml_dtypes.float8_e4m3)
    r = a - hi.astype(np.float32) / s_hi
    lo = np.ascontiguousarray(r * s_lo).astype(ml_dtypes.float8_e4m3)
    return hi, lo


def _make_in_maps(x, norm_weight, w_qkv, w_out):
    x = np.asarray(x, dtype=np.float32)
    norm_weight = np.asarray(norm_weight, dtype=np.float32)
    w_qkv = np.asarray(w_qkv, dtype=np.float32) * norm_weight[None, :]
    w_out = np.asarray(w_out, dtype=np.float32)
    mask_ut = np.triu(np.ones((128, 128), dtype=np.float32))
    eye128 = np.eye(128, dtype=np.float32)
    in_maps = []
    for core in range(8):
        b, g = divmod(core, 4)
        sl = slice(EL * g, EL * (g + 1))
        wq = w_qkv[0 * D:1 * D][sl]
        wk = w_qkv[1 * D:2 * D][sl]
        wv = w_qkv[2 * D:3 * D][sl]
        xh, xl = _hilo(x[b].T, X_S, X_S)
        wqkh, wqkl = _hilo(np.concatenate([wq, wk], axis=0).T, W_S, W_S)
        wvh, wvl = _hilo(wv.T, W_S, W_S)
        in_maps.append({
            "xhT": xh,
            "xlT": xl,
            "wqkhT": wqkh,
            "wqklT": wqkl,
            "wvhT": wvh,
            "wvlT": wvl,
            "woutT": _bf(w_out[:, sl].T),
            "mask": _bf(mask_ut),
            "eye": _bf(eye128),
        })
    return in_maps


def _gather(results):
    out = np.zeros((B, T, D), dtype=np.float32)
    for core in range(8):
        b, _g = divmod(core, 4)
        out[b] += results[core]["outT"].astype(np.float32).T
    return out


def run(x, norm_weight, w_qkv, w_out, trace=False):
    in_maps = _make_in_maps(x, norm_weight, w_qkv, w_out)
    if trace:
        try:
            res = run_bass_kernel_spmd(_get_nc(), in_maps, list(range(8)), trace=True)
            return _gather(res.results), res
        except Exception:
            pass  # NTFF hook unavailable under this axon client; run untraced
    res = run_bass_kernel_spmd(_get_nc(), in_maps, list(range(8)), trace=False)
    return _gather(res.results), res


def kernel(x, norm_weight, w_qkv, w_out):
    out, _res = run(x, norm_weight, w_qkv, w_out)
    return out


# revision 20
# speedup vs baseline: 1.0398x; 1.0398x over previous
"""Trainium2 Bass kernel v2 for MultiHeadSelfAttention (RMSNorm + QKV + causal SDPA + out-proj).

Sharding: 8 cores = batch(2) x head-groups(4); each core: one batch element,
4 of 16 heads (EL=512 d-slice), out-proj partial over its slice, host sums.

v2 changes vs baseline:
  - all matmul operands bf16 (host-cast); fp32 PSUM accumulation. Same PE
    cycles/row as fp32r at >=256 width, but no narrow-tile 4x penalty, half
    the DMA bytes and SBUF footprint.
  - Q/K/V stay SBUF-resident between projection and attention (no DRAM
    round-trip).
  - softmax denominator comes FREE from the AV matmul: AV runs with pt as
    the stationary operand per 128-q block against v_ext [k, dh+1] whose
    last column is ones, so PSUM col 128 accumulates z[q] in exactly the
    per-q-partition orientation the normalize needs (tensor_scalar).
    The old ones-row z matmul (29us of PE) is gone.
  - normalized y [q, dh] blocks are PE-transposed back to [dh, q] for the
    out-projection (cheap: 128 cols each).
  - RMSNorm: norm_weight folded into weights on host; x normalized once
    (xn = x * rsqrt(mean x^2)) on DVE, so q/k/v need no post-scales.
"""

import sys

sys.path.insert(0, '/opt/trn_rl_repo')

import numpy as np
import ml_dtypes

import concourse.bass as bass  # noqa: F401  (import order matters)
from concourse import bacc
import concourse.mybir as mybir
import concourse.tile as tile
from concourse.bass_utils import run_bass_kernel_spmd

B, T, D = 2, 2048, 2048
H_TOT, H_LOC, DH = 16, 4, 128
EL = H_LOC * DH            # 512: local q/k/v width
ND = D // 128              # 16 d-tiles
NT = T // 128              # 16 t-tiles
CH = 512                   # token chunk
NCH = T // CH              # 4 chunks
QT = CH // 128             # 4 q-tiles per chunk
VW = 132                   # v row width: 128 dh + ones col + pad
EPS = 1e-6
F32 = mybir.dt.float32
BF16 = mybir.dt.bfloat16
F8 = mybir.dt.float8e4
X_S = 8.0            # xh = fp8(x * X_S), xl = fp8((x - xh/X_S) * X_S)
W_S = 128.0          # wh = fp8(W * W_S), wl = fp8((W - wh/W_S) * W_S)
P_S = X_S * W_S      # all three residual products share scale P_S = 1024,
                     # so xh@wh + xl@wh + xh@wl accumulate in ONE psum group;
                     # the 1/P_S dequant rides inside the rmsnorm scale
MULT = mybir.AluOpType.mult
SC = float(1.0 / np.sqrt(DH))


def _build():
    nc = bacc.Bacc("TRN2")
    xhT = nc.dram_tensor("xhT", [D, T], F8, kind="ExternalInput")
    xlT = nc.dram_tensor("xlT", [D, T], F8, kind="ExternalInput")
    wqkhT = nc.dram_tensor("wqkhT", [D, 2 * EL], F8, kind="ExternalInput")
    wqklT = nc.dram_tensor("wqklT", [D, 2 * EL], F8, kind="ExternalInput")
    wvhT = nc.dram_tensor("wvhT", [D, EL], F8, kind="ExternalInput")
    wvlT = nc.dram_tensor("wvlT", [D, EL], F8, kind="ExternalInput")
    woutT = nc.dram_tensor("woutT", [EL, D], BF16, kind="ExternalInput")
    mask = nc.dram_tensor("mask", [128, 128], BF16, kind="ExternalInput")
    eye = nc.dram_tensor("eye", [128, 128], BF16, kind="ExternalInput")
    outT = nc.dram_tensor("outT", [D, T], BF16, kind="ExternalOutput")

    with tile.TileContext(nc) as tc:
        with tc.tile_pool(name="const", bufs=1) as cst, \
             tc.tile_pool(name="dram", bufs=1, space="DRAM") as dramp, \
             tc.tile_pool(name="persist", bufs=1) as pers:
            mask_sb = cst.tile([128, 128], BF16)
            eye_sb = cst.tile([128, 128], BF16)
            ones = cst.tile([128, 8], BF16)
            nc.gpsimd.memset(ones[:], 1.0)
            eps_sb = cst.tile([1, 1], F32)
            nc.gpsimd.memset(eps_sb[:], EPS * P_S * P_S)

            wqkh_sb = pers.tile([128, ND // 2, 2, 2 * EL], F8)
            wqkl_sb = pers.tile([128, ND // 2, 2, 2 * EL], F8)
            wvh_sb = pers.tile([128, ND // 2, 2, EL], F8)
            wvl_sb = pers.tile([128, ND // 2, 2, EL], F8)
            wout_sb = pers.tile([128, H_LOC, D], BF16)
            qT_sb = pers.tile([128, H_LOC, T], BF16)
            kT_sb = pers.tile([128, H_LOC, T], BF16)
            v_sb = pers.tile([128, NT, H_LOC, VW], BF16)
            nc.gpsimd.memset(v_sb[:, :, :, 128:129], 1.0)

            def load_weights_qk():
                # i-row-block granularity: full-width 1024B runs (no narrow-DMA
                # penalty) and QKV(0) i-ascending matmuls start on block 0
                for g in range(4):
                    nc.sync.dma_start(
                        wqkh_sb[:, 2 * g:2 * g + 2, :, :],
                        wqkhT[g * 512:(g + 1) * 512, :].rearrange("(i two p) e -> p i two e", p=128, two=2))

            def load_weights_qkl():
                for g in range(4):
                    nc.sync.dma_start(
                        wqkl_sb[:, 2 * g:2 * g + 2, :, :],
                        wqklT[g * 512:(g + 1) * 512, :].rearrange("(i two p) e -> p i two e", p=128, two=2))

            def load_weights_rest():
                nc.sync.dma_start(wvh_sb[:], wvhT[0:D, :].rearrange("(i two p) e -> p i two e", p=128, two=2))
                nc.sync.dma_start(wvl_sb[:], wvlT[0:D, :].rearrange("(i two p) e -> p i two e", p=128, two=2))

            def load_wout():
                nc.sync.dma_start(wout_sb[:], woutT[0:EL, :].rearrange("(dl p) e -> p dl e", p=128))
                nc.sync.dma_start(mask_sb[:], mask[:, :])
                nc.sync.dma_start(eye_sb[:], eye[:, :])

            # ---------------- Phase A: RMSNorm + QKV projection ----------------
            with tc.tile_pool(name="xa", bufs=2) as xa_p, \
                 tc.tile_pool(name="pa_sb", bufs=2) as pa_sb, \
                 tc.tile_pool(name="pa_ps", bufs=2, space="PSUM") as pa_ps, \
                 tc.tile_pool(name="pa_ps1", bufs=1, space="PSUM") as pa_ps1:
                xc_tiles = {}
                xn_tiles = {}
                sq_tiles = {}
                sbc_tiles = {}

                def emit_load(c, parts=(ND // 2,), defer_xl=False):
                    xh = xa_p.tile([128, ND // 2, 2, CH], F8, tag="xh")
                    xl = xa_p.tile([128, ND // 2, 2, CH], F8, tag="xl")
                    k0 = 0
                    for n in parts:
                        nc.sync.dma_start(
                            xh[:, k0:k0 + n, :, :],
                            xhT[k0 * 256:(k0 + n) * 256,
                                c * CH:(c + 1) * CH].rearrange("(i two p) t -> p i two t", p=128, two=2))
                        k0 += n
                    xc_tiles[c] = (xh, xl)

                    def xl_load():
                        nc.sync.dma_start(
                            xl[:, :, :, :],
                            xlT[0:D, c * CH:(c + 1) * CH].rearrange("(i two p) t -> p i two t", p=128, two=2))
                    if defer_xl:
                        return xl_load
                    xl_load()

                def emit_sq(c):
                    xh, _xl = xc_tiles[c]
                    sqs = []
                    for i in range(ND // 2):
                        sq = pa_sb.tile([128, 2, CH], BF16, tag="sq", bufs=4)
                        src = xh[:, i, :, :]
                        if i % 2 == 0:
                            nc.scalar.square(sq[:], src)
                        else:
                            nc.vector.tensor_tensor(sq[:], src, src, MULT)
                        sqs.append(sq)
                    sq_tiles[c] = sqs

                def emit_ssq_chain(c, preload_exp=False):
                    sqs = sq_tiles.pop(c)
                    ssq = pa_ps1.tile([1, CH], F32, tag="ssq")
                    for kd in range(ND):
                        nc.tensor.matmul(ssq[:], ones[:, 0:1], sqs[kd // 2][:, kd % 2, :],
                                         start=(kd == 0), stop=(kd == ND - 1))
                    tmp_s = pa_sb.tile([1, CH], F32, tag="tmp_s")
                    nc.scalar.activation(tmp_s[:], ssq[:], mybir.ActivationFunctionType.Sqrt,
                                         bias=eps_sb[:], scale=P_S * P_S / (X_S * X_S * D))
                    if preload_exp:
                        # pull the Exp act-table load into Phase A slack so the
                        # first Phase B exp doesn't pay the 1.3us switch
                        dummy = pa_sb.tile([1, 1], F32, tag="expdummy")
                        nc.scalar.activation(dummy[:], eps_sb[:],
                                             mybir.ActivationFunctionType.Exp)
                    s_row = pa_sb.tile([1, CH], F32, tag="s_row")
                    nc.vector.reciprocal(s_row[:], tmp_s[:])
                    s_bf = pa_sb.tile([1, CH], BF16, tag="s_bf")
                    nc.vector.tensor_copy(s_bf[:], s_row[:])
                    sb_c = pa_sb.tile([128, CH], BF16, tag="sb_c")
                    nc.gpsimd.partition_broadcast(sb_c[:], s_bf[:])
                    # SBUF->SBUF scatter DMA misbehaves on hw; bounce via DRAM
                    s_d = dramp.tile([1, CH], F32, tag="s_d", bufs=2)
                    nc.sync.dma_start(s_d[:], s_row[:])
                    s_col = pa_sb.tile([128, QT], F32, tag="s_col")
                    nc.sync.dma_start(s_col[:], s_d[0:1, :].rearrange("o (tt p) -> p (o tt)", p=128))
                    sbc_tiles[c] = (sb_c, s_col)

                def emit_qkv(c, mid_hook=None, tail_hook=None):
                    xh, xl = xc_tiles.pop(c)
                    sb_c, s_col = sbc_tiles.pop(c)
                    DR = mybir.MatmulPerfMode.DoubleRow
                    NP = ND // 2

                    def residual_mm(g, et_sl, moving_of):
                        for i in range(NP):
                            nc.tensor.matmul(g, wqkh_sb[:, i, :, et_sl], moving_of(xh, i),
                                             perf_mode=DR, start=(i == 0), stop=False)
                        for i in range(NP):
                            nc.tensor.matmul(g, wqkl_sb[:, i, :, et_sl], moving_of(xh, i),
                                             perf_mode=DR, start=False, stop=False)
                        for i in range(NP):
                            nc.tensor.matmul(g, wqkh_sb[:, i, :, et_sl], moving_of(xl, i),
                                             perf_mode=DR, start=False, stop=(i == NP - 1))

                    for et in range(8):   # 4 q tiles then 4 k tiles
                        if et == 4 and mid_hook is not None:
                            mid_hook()
                        esl = slice(et * 128, (et + 1) * 128)
                        g = pa_ps.tile([128, CH], F32, tag="g", bufs=3)
                        residual_mm(g[:], esl, lambda t, i: t[:, i, :, :])
                        dst = qT_sb if et < 4 else kT_sb
                        nc.vector.tensor_tensor(dst[:, et % 4, c * CH:(c + 1) * CH], g[:], sb_c[:], MULT)
                    for tt in range(QT):
                        j = c * QT + tt
                        tsl = slice(tt * 128, (tt + 1) * 128)
                        g = pa_ps.tile([128, EL], F32, tag="g", bufs=3)
                        for i in range(NP):
                            nc.tensor.matmul(g[:], xh[:, i, :, tsl], wvh_sb[:, i, :, :],
                                             perf_mode=DR, start=(i == 0), stop=False)
                        for i in range(NP):
                            nc.tensor.matmul(g[:], xl[:, i, :, tsl], wvh_sb[:, i, :, :],
                                             perf_mode=DR, start=False, stop=False)
                        for i in range(NP):
                            nc.tensor.matmul(g[:], xh[:, i, :, tsl], wvl_sb[:, i, :, :],
                                             perf_mode=DR, start=False, stop=(i == NP - 1))
                        for h in range(H_LOC):
                            if c == NCH - 1:
                                nc.vector.tensor_scalar_mul(v_sb[:, j, h, 0:128],
                                                            g[:, h * 128:(h + 1) * 128],
                                                            s_col[:, tt:tt + 1])
                            else:
                                nc.scalar.activation(v_sb[:, j, h, 0:128], g[:, h * 128:(h + 1) * 128],
                                                     mybir.ActivationFunctionType.Copy,
                                                     scale=s_col[:, tt:tt + 1])
                    if tail_hook is not None:
                        tail_hook()

                xl0_load = emit_load(0, parts=(1, 1, 2, 4), defer_xl=True)
                load_weights_qk()
                load_weights_qkl()
                xl0_load()
                load_weights_rest()
                emit_load(1)
                load_wout()
                emit_sq(0)
                emit_ssq_chain(0)
                for c in range(NCH):
                    def mid():
                        if c + 1 < NCH:
                            emit_ssq_chain(c + 1, preload_exp=(c + 1 == NCH - 1))
                            if c + 2 < NCH:
                                emit_load(c + 2)

                    def tail():
                        pass
                    if c + 1 < NCH:
                        emit_sq(c + 1)
                    emit_qkv(c, mid_hook=mid, tail_hook=tail)

            # ---------------- Phase B: causal attention + out-projection ----------------
            with tc.tile_pool(name="pb_sb", bufs=2) as pb_sb, \
                 tc.tile_pool(name="pb_ps", bufs=2, space="PSUM") as pb_ps:

                def emit_scores(c, h):
                    """score matmuls + exp + mask for (chunk c, head h); returns pt tiles."""
                    jmax = (c + 1) * QT
                    pts = []
                    for j in range(jmax):
                        off = (j - c * QT) * 128 if j >= c * QT else 0
                        st_ps = pb_ps.tile([128, CH], F32, tag="st_ps", bufs=3)
                        nc.tensor.matmul(st_ps[:, off:], kT_sb[:, h, j * 128:(j + 1) * 128],
                                         qT_sb[:, h, c * CH + off:(c + 1) * CH], start=True, stop=True)
                        pt = pb_sb.tile([128, CH], BF16, tag="pt", bufs=20)
                        nc.scalar.activation(pt[:, off:], st_ps[:, off:],
                                             mybir.ActivationFunctionType.Exp, scale=SC)
                        if j >= c * QT:
                            nc.vector.tensor_tensor(pt[:, off:off + 128], pt[:, off:off + 128],
                                                    mask_sb[:], MULT)
                        pts.append(pt)
                    return pts

                def emit_av(c, h, pts, y_sb):
                    """AV with stationary pt per q-block; v_ext ones col gives z in PSUM
                    col 128; then normalize on DVE. Returns yn tiles for later transpose."""
                    yT_a = pb_ps.tile([128, 2, 129], F32, tag="yT_a", bufs=1)
                    yT_b = pb_ps.tile([128, 2, 129], F32, tag="yT_b", bufs=1)
                    halves = [yT_a, yT_b]
                    # accumulate with start=False onto memset-zeroed PSUM: four
                    # interleaved q-block groups share banks, and start=True would
                    # zero the whole 2KB zero-region (clobbering neighbors)
                    nc.vector.memset(yT_a[:], 0.0)
                    nc.vector.memset(yT_b[:], 0.0)
                    for j in range((c + 1) * QT):
                        off = (j - c * QT) * 128 if j >= c * QT else 0
                        pt = pts[j]
                        for qb in range(QT):
                            if qb * 128 < off:
                                continue
                            nc.tensor.matmul(halves[qb // 2][:, qb % 2, :],
                                             pt[:, qb * 128:(qb + 1) * 128],
                                             v_sb[:, j, h, 0:129],
                                             start=False, stop=(j == c * QT + qb),
                                             skip_group_check=True)
                    yns = []
                    for qb in range(QT):
                        yT = halves[qb // 2][:, qb % 2, :]
                        rz = pb_sb.tile([128, 1], F32, tag="rz", bufs=4)
                        nc.vector.reciprocal(rz[:], yT[:, 128:129])
                        yn = pb_sb.tile([128, 128], BF16, tag="yn", bufs=8)
                        nc.vector.tensor_scalar_mul(yn[:], yT[:, 0:128], rz[:])
                        yns.append(yn)
                    return yns

                def emit_transposes(c, h, yns, y_sb):
                    for qb in range(QT):
                        tr_ps = pb_ps.tile([128, 128], BF16, tag="tr_ps", bufs=1)
                        nc.tensor.transpose(tr_ps[:], yns[qb][:], eye_sb[:])
                        nc.vector.tensor_copy(y_sb[:, h, qb * 128:(qb + 1) * 128], tr_ps[:])

                def emit_outproj_part(c, y_sb, eo_list):
                    for eo in eo_list:
                        o_ps = pb_ps.tile([128, CH], F32, tag="o_ps", bufs=2)
                        for dl in range(H_LOC):
                            nc.tensor.matmul(o_ps[:], wout_sb[:, dl, eo * 128:(eo + 1) * 128],
                                             y_sb[:, dl, :], start=(dl == 0), stop=(dl == H_LOC - 1))
                        o_sb = pb_sb.tile([128, CH], BF16, tag="o_sb", bufs=4)
                        if eo % 2 == 0:
                            nc.vector.tensor_copy(o_sb[:], o_ps[:])
                        else:
                            nc.scalar.activation(o_sb[:], o_ps[:], mybir.ActivationFunctionType.Copy)
                        nc.sync.dma_start(outT[eo * 128:(eo + 1) * 128, c * CH:(c + 1) * CH], o_sb[:])

                # slot schedule per chunk: scores(h) | AV(h-1)+norm | transposes(h-2)
                # | previous chunk's out-proj spread across slots
                prev_y = None
                for c in range(NCH):
                    y_sb = pb_sb.tile([128, H_LOC, CH], BF16, tag="y_sb", bufs=2)
                    pend_av = None   # (h, pts)
                    pend_tr = None   # (h, yns)
                    for h in range(H_LOC):
                        pts = emit_scores(c, h)
                        if prev_y is not None:
                            eos = [range(0, 8), range(8, 12), range(12, 16), range(0, 0)][h]
                            emit_outproj_part(c - 1, prev_y, eos)
                        if pend_av is not None:
                            yns = emit_av(c, pend_av[0], pend_av[1], y_sb)
                            if pend_tr is not None:
                                emit_transposes(c, pend_tr[0], pend_tr[1], y_sb)
                            pend_tr = (pend_av[0], yns)
                        pend_av = (h, pts)
                    yns = emit_av(c, pend_av[0], pend_av[1], y_sb)
                    if pend_tr is not None:
                        emit_transposes(c, pend_tr[0], pend_tr[1], y_sb)
                    emit_transposes(c, pend_av[0], yns, y_sb)
                    prev_y = y_sb
                emit_outproj_part(NCH - 1, prev_y, range(16))
    nc.finalize()
    return nc


_BUILT = None


def _get_nc():
    global _BUILT
    if _BUILT is None:
        _BUILT = _build()
    return _BUILT


def _bf(a):
    return np.ascontiguousarray(a).astype(ml_dtypes.bfloat16)


def _hilo(a, s_hi, s_lo):
    hi = np.ascontiguousarray(a * s_hi).astype(ml_dtypes.float8_e4m3)
    r = a - hi.astype(np.float32) / s_hi
    lo = np.ascontiguousarray(r * s_lo).astype(ml_dtypes.float8_e4m3)
    return hi, lo


def _make_in_maps(x, norm_weight, w_qkv, w_out):
    x = np.asarray(x, dtype=np.float32)
    norm_weight = np.asarray(norm_weight, dtype=np.float32)
    w_qkv = np.asarray(w_qkv, dtype=np.float32) * norm_weight[None, :]
    w_out = np.asarray(w_out, dtype=np.float32)
    mask_ut = np.triu(np.ones((128, 128), dtype=np.float32))
    eye128 = np.eye(128, dtype=np.float32)
    in_maps = []
    for core in range(8):
        b, g = divmod(core, 4)
        sl = slice(EL * g, EL * (g + 1))
        wq = w_qkv[0 * D:1 * D][sl]
        wk = w_qkv[1 * D:2 * D][sl]
        wv = w_qkv[2 * D:3 * D][sl]
        xh, xl = _hilo(x[b].T, X_S, X_S)
        wqkh, wqkl = _hilo(np.concatenate([wq, wk], axis=0).T, W_S, W_S)
        wvh, wvl = _hilo(wv.T, W_S, W_S)
        in_maps.append({
            "xhT": xh,
            "xlT": xl,
            "wqkhT": wqkh,
            "wqklT": wqkl,
            "wvhT": wvh,
            "wvlT": wvl,
            "woutT": _bf(w_out[:, sl].T),
            "mask": _bf(mask_ut),
            "eye": _bf(eye128),
        })
    return in_maps


def _gather(results):
    out = np.zeros((B, T, D), dtype=np.float32)
    for core in range(8):
        b, _g = divmod(core, 4)
        out[b] += results[core]["outT"].astype(np.float32).T
    return out


def run(x, norm_weight, w_qkv, w_out, trace=False):
    in_maps = _make_in_maps(x, norm_weight, w_qkv, w_out)
    if trace:
        try:
            res = run_bass_kernel_spmd(_get_nc(), in_maps, list(range(8)), trace=True)
            return _gather(res.results), res
        except Exception:
            pass  # NTFF hook unavailable under this axon client; run untraced
    res = run_bass_kernel_spmd(_get_nc(), in_maps, list(range(8)), trace=False)
    return _gather(res.results), res


def kernel(x, norm_weight, w_qkv, w_out):
    out, _res = run(x, norm_weight, w_qkv, w_out)
    return out
